# revision 22
# baseline (speedup 1.0000x reference)
"""Bi-attention kernel for Trainium2 (8 NeuronCores, data-parallel over batch).

Per-core computation (B=1 slice, Lc=512, Lq=64, D=256):
  score[i,j] = sum_d c[i,d] q[j,d] w_p[d] + rv_j,  rv_j = q_j.w_q + b - 1e30*(1-mask_j)
  h = softmax_j(score);  U[i] = sum_j h[i,j] * (q_j.w_mem)
  m_i = max_j score[i,j] + c_i.w_c;  u = softmax_i(m);  H = sum_i u[i]*ctx1[i]
  ctx1 = c.w_in;  G[i] = [ctx1[i], U[i], ctx1[i]*U[i], U[i]*H]
(The c_i.w_c term is constant per row i, so it cancels inside softmax_j; it
enters only through e^{m_i} = (max_j e^{score_ij}) * e^{c_i.w_c}.)

Schedule (tuned against the CoreSim legacy cost model; GPSIMD never touches
PSUM, which the BIR verifier enforces):
  - 3 input DMAs (two contextT halves + one packed bf16 param tensor) are
    issued at t~100 on the three DMA-capable queues (SP/Act/Pool); each DMA
    costs 500ns of engine time + ~1717ns fixed latency -> inputs land ~2317ns.
  - sq = q.w_q and q1 = q.w_mem are computed on Pool as per-partition
    multiplies + partition_all_reduce (which also yields the q1 broadcast
    for free), keeping the whole rv chain off PSUM and off the PE.
  - All matmuls run in bf16 (1 cycle/row vs 4 for fp32). Scores accumulate
    into two half PSUM tiles [128, 2*66]; only one accumulation group can be
    open per tile, so chunk order is c0,c2 (both tiles) -> c0 bias close as
    rv lands -> c1 -> close c1 (unblocks exp half 0) -> c2 bias, c3.
  - Exp runs in 2 halves on Act; row-sum (den) and row-max (M, of E) via
    cheap Pool binary trees; the U numerator via DVE tensor_tensor_reduce.
  - The u-softmax partition sums + broadcast are one Pool
    partition_all_reduce; the whole tail stays on Pool (no sem ping-pong).
  - Output is a single DMA issued from the idle SP queue.
"""

import sys

for _p in ("/opt/trn_rl_repo", "/root/.axon_site/_ro/trn_rl_repo"):
    if _p not in sys.path:
        sys.path.append(_p)

import numpy as np
import ml_dtypes

import concourse.bacc as bacc
import concourse.bass as bass
import concourse.bass_isa as bass_isa
import concourse.tile as tile
from concourse import mybir
from concourse.bass_utils import run_bass_kernel_spmd

B, LC, LQ, D = 8, 512, 64, 256
NEG_BIG = 1e30
NCHUNK = LC // 128  # 4 chunks of 128 context rows
KD = D // 128  # 2 contraction chunks
GW = LQ + 2  # psum group width: 64 scores | c.w_c | c.w_in
F32 = mybir.dt.float32
BF16 = mybir.dt.bfloat16
AF = mybir.ActivationFunctionType
ALU = mybir.AluOpType
AX = mybir.AxisListType
BF = ml_dtypes.bfloat16

# packed bf16 params column layout
PC_QT = 0  # cols 0:128    qT chunks (64 cols each)
PC_WF = 128  # cols 128:134  [w_q k0,k1 | w_mem k0,k1 | w_p k0,k1]
PC_WC = 134  # cols 134:136  w_c chunks
PC_WIN = 136  # cols 136:138  w_in chunks
PC_B = 138  # col 138       att_b at row 0
PC_MK = 139  # cols 139:203  maskt = -1e30*(1-mask) at row 0
NPB = PC_MK + LQ


def build_nc():
    nc = bacc.Bacc("TRN2", target_bir_lowering=False, debug=False)

    ctxb_d = nc.dram_tensor("ctxb", [128, KD, LC], BF16, kind="ExternalInput")
    parb_d = nc.dram_tensor("parb", [128, NPB], BF16, kind="ExternalInput")
    g_d = nc.dram_tensor("G", [LC, 4], F32, kind="ExternalOutput")

    with tile.TileContext(nc) as tc:
        with (
            tc.tile_pool(name="singles", bufs=1) as singles,
            tc.tile_pool(name="ps_sc", bufs=2, space="PSUM") as ps_sc,
            tc.tile_pool(name="ps_cw", bufs=1, space="PSUM") as ps_cw,
        ):
            # ---- input DMAs. Pool-engine consumers observe DMA data right
            # after the issuing engine's cost window (no +1717ns DMA latency),
            # so parb goes on Pool's own queue and the context comes in one SP
            # DMA that Pool immediately relays into cb2 for the PE (which
            # would otherwise stall on the DMA semaphore until ~2600ns).
            parb = singles.tile([128, NPB], BF16)
            nc.gpsimd.dma_start(out=parb, in_=parb_d[:, :])
            cb = singles.tile([128, KD * LC], BF16)
            nc.sync.dma_start(
                out=cb, in_=ctxb_d.rearrange("p k l -> p (k l)")
            )
            cbv = cb.rearrange("p (k l) -> p k l", k=KD)

            # ---- constants (Pool) + act table warm (Act) ----
            ones1 = singles.tile([1, 128], BF16)
            nc.gpsimd.memset(ones1, 1.0)
            rv66 = singles.tile([1, GW], BF16)
            nc.gpsimd.memset(rv66, 0.0)
            # dummy early activation: pulls the 1283ns act-table load to t~200
            # (it is inserted before the first Activation in queue order)
            warm1 = singles.tile([1, 1], F32)
            nc.gpsimd.memset(warm1, 0.0)
            warmo = singles.tile([1, 1], F32)
            nc.scalar.activation(warmo, warm1, AF.Exp)

            def qt(k):
                return parb[:, PC_QT + LQ * k : PC_QT + LQ * (k + 1)]

            # scalar-ptr operands must be fp32: convert small weights first
            wf = singles.tile([128, 6], F32)
            nc.gpsimd.tensor_copy(wf, parb[:, PC_WF : PC_WF + 6])
            bf1 = singles.tile([1, 1], F32)
            nc.gpsimd.tensor_copy(bf1, parb[0:1, PC_B : PC_B + 1])

            # ---- rhsA_k [128, 66] = [w_p*qT | w_c | w_in] (Pool) ----
            rhsA = []
            for k in range(KD):
                rhsA_k = singles.tile([128, GW], BF16, tag=f"rhsA{k}", name=f"rhsA{k}")
                nc.gpsimd.tensor_scalar_mul(
                    rhsA_k[:, 0:LQ], qt(k), wf[:, 4 + k : 5 + k]
                )
                nc.gpsimd.tensor_copy(
                    rhsA_k[:, LQ : LQ + 1], parb[:, PC_WC + k : PC_WC + k + 1]
                )
                nc.gpsimd.tensor_copy(
                    rhsA_k[:, LQ + 1 : LQ + 2], parb[:, PC_WIN + k : PC_WIN + k + 1]
                )
                rhsA.append(rhsA_k)

            # ---- sq / q1 via Pool partition_all_reduce (no PE, no PSUM) ----
            tq = singles.tile([128, LQ], F32)
            sq_bc = singles.tile([128, LQ], F32)
            nc.gpsimd.tensor_scalar_mul(tq, qt(0), wf[:, 0:1])
            nc.gpsimd.scalar_tensor_tensor(
                tq, qt(1), wf[:, 1:2], tq, op0=ALU.mult, op1=ALU.add
            )
            nc.gpsimd.partition_all_reduce(sq_bc, tq, 128, bass_isa.ReduceOp.add)
            # rv = sq + b + maskt (all SBUF, bf16 out)
            nc.gpsimd.scalar_tensor_tensor(
                rv66[0:1, 0:LQ],
                sq_bc[0:1, :],
                bf1,
                parb[0:1, PC_MK : PC_MK + LQ],
                op0=ALU.add,
                op1=ALU.add,
            )
            tq2 = singles.tile([128, LQ], F32)
            q1bc = singles.tile([128, LQ], BF16)
            nc.gpsimd.tensor_scalar_mul(tq2, qt(0), wf[:, 2:3])
            nc.gpsimd.scalar_tensor_tensor(
                tq2, qt(1), wf[:, 3:4], tq2, op0=ALU.mult, op1=ALU.add
            )
            nc.gpsimd.partition_all_reduce(q1bc, tq2, 128, bass_isa.ReduceOp.add)

            # ---- scores: two half PSUM tiles [128, 2*66] ----
            Sh = []
            for h in range(2):
                S = ps_sc.tile([128, 2 * GW], F32, tag="score", name=f"S{h}")
                Sh.append(S.rearrange("p (c w) -> p c w", c=2))

            def kmm(c, k):
                h, cc = divmod(c, 2)
                nc.tensor.matmul(
                    Sh[h][:, cc, :],
                    cbv[:, k, 128 * c : 128 * (c + 1)],
                    rhsA[k],
                    start=(k == 0),
                    stop=False,
                )

            def bmm(c):
                h, cc = divmod(c, 2)
                nc.tensor.matmul(Sh[h][:, cc, :], ones1, rv66, start=False, stop=True)

            kmm(0, 0)
            kmm(0, 1)
            kmm(2, 0)
            kmm(2, 1)
            bmm(0)
            kmm(1, 0)
            kmm(1, 1)
            bmm(1)
            bmm(2)
            kmm(3, 0)
            kmm(3, 1)
            bmm(3)

            # c.w_c / c.w_in in their own PSUM tile (8 tiny N=2 matmuls) so
            # the S tiles have no readers besides the exps (PSUM readers
            # serialize, and the cw extraction must not gate the exps).
            cwps = ps_cw.tile([128, NCHUNK, 2], F32, tag="cwps")
            for c in range(NCHUNK):
                for k in range(KD):
                    nc.tensor.matmul(
                        cwps[:, c, :],
                        cbv[:, k, 128 * c : 128 * (c + 1)],
                        rhsA[k][:, LQ : LQ + 2],
                        start=(k == 0),
                        stop=(k == KD - 1),
                    )
            cw = singles.tile([128, NCHUNK, 2], F32)
            nc.vector.tensor_copy(cw, cwps)

            G = singles.tile([128, NCHUNK, 4], F32)

            def gcol(j):
                return G[:, :, j : j + 1].rearrange("p c o -> p (c o)")

            cwc4 = cw[:, :, 0:1].rearrange("p c o -> p (c o)")
            nc.gpsimd.tensor_copy(gcol(0), cw[:, :, 1:2].rearrange("p c o -> p (c o)"))

            # ---- softmax_j pipeline over halves ----
            E = singles.tile([128, NCHUNK, LQ], BF16)
            den4 = singles.tile([128, NCHUNK], F32)
            M4 = singles.tile([128, NCHUNK], BF16)
            num4 = singles.tile([128, NCHUNK], F32)
            prods = singles.tile([128, LQ], BF16)
            dscr = singles.tile([128, 2, 32], F32)
            mscr = singles.tile([128, 2, 32], BF16)

            def pool_tree(dst2, src3, scr, op):
                # src3: [128, 2, 64] -> dst2: [128, 2] via contiguous halving
                nc.gpsimd.tensor_tensor(scr, src3[:, :, 0:32], src3[:, :, 32:64], op)
                w = 16
                while w >= 1:
                    a = scr[:, :, 0:w]
                    if w == 1:
                        a = dst2.rearrange("p (c o) -> p c o", o=1)
                    nc.gpsimd.tensor_tensor(a, scr[:, :, 0:w], scr[:, :, w : 2 * w], op)
                    w //= 2

            for h in range(2):
                nc.scalar.activation(
                    E[:, 2 * h : 2 * h + 2, :], Sh[h][:, :, 0:LQ], AF.Exp
                )
            for h in range(2):
                Eh = E[:, 2 * h : 2 * h + 2, :]
                pool_tree(den4[:, 2 * h : 2 * h + 2], Eh, dscr, ALU.add)
                pool_tree(M4[:, 2 * h : 2 * h + 2], Eh, mscr, ALU.max)
            for c in range(NCHUNK):
                nc.vector.tensor_tensor_reduce(
                    out=prods,
                    in0=E[:, c, :],
                    in1=q1bc,
                    scale=1.0,
                    scalar=0.0,
                    op0=ALU.mult,
                    op1=ALU.add,
                    accum_out=num4[:, c : c + 1],
                )

            # ---- u softmax over i, on Pool after one small Act exp ----
            ecwc4 = singles.tile([128, NCHUNK], F32)
            nc.scalar.activation(ecwc4, cwc4, AF.Exp)
            em4 = singles.tile([128, NCHUNK], F32)
            emc4 = singles.tile([128, NCHUNK], F32)
            em2 = singles.tile([128, 2], F32)
            em2a = singles.tile([128, 2], F32)
            t2 = singles.tile([128, 2], F32)
            nc.gpsimd.tensor_tensor(em4, M4, ecwc4, ALU.mult)
            nc.gpsimd.tensor_tensor(emc4, em4, gcol(0), ALU.mult)
            nc.gpsimd.tensor_tensor(t2, em4[:, 0:2], em4[:, 2:4], ALU.add)
            nc.gpsimd.tensor_tensor(em2[:, 0:1], t2[:, 0:1], t2[:, 1:2], ALU.add)
            nc.gpsimd.tensor_tensor(t2, emc4[:, 0:2], emc4[:, 2:4], ALU.add)
            nc.gpsimd.tensor_tensor(em2[:, 1:2], t2[:, 0:1], t2[:, 1:2], ALU.add)
            nc.gpsimd.partition_all_reduce(em2a, em2, 128, bass_isa.ReduceOp.add)
            h1 = singles.tile([128, 1], F32)
            nc.gpsimd.tensor_tensor(h1, em2a[:, 1:2], em2a[:, 0:1], ALU.divide)

            nc.gpsimd.tensor_tensor(gcol(1), num4, den4, ALU.divide)  # U
            nc.gpsimd.tensor_tensor(gcol(2), gcol(0), gcol(1), ALU.mult)
            nc.gpsimd.tensor_scalar_mul(gcol(3), gcol(1), h1)

            nc.sync.dma_start(out=g_d.rearrange("(c p) g -> p c g", p=128), in_=G)

    nc.finalize()
    return nc


_NC = None


def _get_nc():
    global _NC
    if _NC is None:
        _NC = build_nc()
    return _NC


def pack_params(att_w, att_b, w_in, w_mem, mask_b, question_b):
    par = np.zeros((128, NPB), np.float32)
    qt = question_b.T.reshape(KD, 128, LQ)
    for k in range(KD):
        par[:, PC_QT + LQ * k : PC_QT + LQ * (k + 1)] = qt[k]
        par[:, PC_WF + k] = att_w[256 + 128 * k : 256 + 128 * (k + 1)]  # w_q
        par[:, PC_WF + 2 + k] = w_mem[128 * k : 128 * (k + 1)]
        par[:, PC_WF + 4 + k] = att_w[512 + 128 * k : 512 + 128 * (k + 1)]  # w_p
        par[:, PC_WC + k] = att_w[128 * k : 128 * (k + 1)]
        par[:, PC_WIN + k] = w_in[128 * k : 128 * (k + 1)]
    par[0, PC_B] = att_b[0]
    par[0, PC_MK : PC_MK + LQ] = -NEG_BIG * (1.0 - mask_b.astype(np.float32))
    return par.astype(BF)


def make_in_maps(context, question, mask, att_w, att_b, w_in, w_mem):
    context = np.asarray(context, np.float32)
    question = np.asarray(question, np.float32)
    mask = np.asarray(mask, np.int32)
    att_w = np.asarray(att_w, np.float32)
    att_b = np.asarray(att_b, np.float32)
    w_in = np.asarray(w_in, np.float32)
    w_mem = np.asarray(w_mem, np.float32)
    maps = []
    for b in range(B):
        # ctxb[p, k, i] = context[b, i, 128*k + p]
        ctxb = np.ascontiguousarray(
            np.moveaxis(context[b].T.reshape(KD, 128, LC), 0, 1)
        ).astype(BF)
        maps.append(
            {
                "ctxb": ctxb,
                "parb": pack_params(att_w, att_b, w_in, w_mem, mask[b], question[b]),
            }
        )
    return maps


def kernel(context, question, mask, att_w, att_b, w_in, w_mem):
    nc = _get_nc()
    in_maps = make_in_maps(context, question, mask, att_w, att_b, w_in, w_mem)
    res = run_bass_kernel_spmd(nc, in_maps, core_ids=list(range(B)))
    return np.stack([res.results[c]["G"] for c in range(B)], axis=0)


# revision 23
# speedup vs baseline: 1.1174x; 1.1174x over previous
"""Bi-attention kernel for Trainium2 (8 NeuronCores, data-parallel over batch).

Per-core computation (B=1 slice, Lc=512, Lq=64, D=256):
  score[i,j] = sum_d c[i,d] q[j,d] w_p[d] + rv_j,  rv_j = q_j.w_q + b - 1e30*(1-mask_j)
  h = softmax_j(score);  U[i] = sum_j h[i,j] * (q_j.w_mem)
  m_i = max_j score[i,j] + c_i.w_c;  u = softmax_i(m);  H = sum_i u[i]*ctx1[i]
  ctx1 = c.w_in;  G[i] = [ctx1[i], U[i], ctx1[i]*U[i], U[i]*H]
(c_i.w_c cancels inside softmax_j; it enters only via
 e^{m_i} = (max_j e^{score_ij}) * e^{c_i.w_c}.)

Schedule, tuned against the CoreSim legacy cost model:
  - DMA data is visible to the ISSUING engine's later ops right after the
    DMA's engine-cost window, but cross-engine consumers wait the full
    ~1717ns (HWDGE) / ~1883ns (Pool SWDGE) completion latency. So Pool
    issues the params DMA + a context DMA for row-chunks 0-2 on its own
    queue, runs the whole param chain at ~1200ns, and relays the context
    into SBUF tiles ([128,128] copies, 107ns each) that the PE consumes at
    Pool-sem latency (~1700-2200ns instead of 2600+). Chunk 3 comes on SP
    (PE reads it directly at ~2500ns), which overlaps the chunk 0-2 pipe.
  - All matmuls in bf16 (1 cycle/row). Scores accumulate in two half PSUM
    tiles [128, 2*66] (one accumulation group open per tile); the rv bias
    is a K=1 ones-row matmul per chunk that also stops the group.
  - Exp in 2 halves on Act (the 1283ns act-table load is pulled to t~200
    by a dummy warm activation); row-sum/row-max of E via Pool binary
    trees; U numerator via DVE tensor_tensor_reduce; c.w_c / c.w_in get
    their own tiny PSUM tile (8 N=2 matmuls) so the exps are the score
    tiles' only readers (PSUM readers serialize).
  - sq/q1 and the u-softmax partition sums use Pool partition_all_reduce
    (no PE round-trip, no PSUM); the whole tail stays on Pool.
  - Output: one DMA from the idle SP queue; total = issue + 2817ns.
"""

import sys

for _p in ("/opt/trn_rl_repo", "/root/.axon_site/_ro/trn_rl_repo"):
    if _p not in sys.path:
        sys.path.append(_p)

import numpy as np
import ml_dtypes

import concourse.bacc as bacc
import concourse.bass as bass
import concourse.bass_isa as bass_isa
import concourse.tile as tile
from concourse import mybir
from concourse.bass_utils import run_bass_kernel_spmd

B, LC, LQ, D = 8, 512, 64, 256
NEG_BIG = 1e30
NCHUNK = LC // 128  # 4 chunks of 128 context rows
NCA = 3  # chunks 0..2 arrive via Pool and are relayed
KD = D // 128  # 2 contraction chunks
GW = LQ + 2  # psum group width: 64 scores | c.w_c | c.w_in
F32 = mybir.dt.float32
BF16 = mybir.dt.bfloat16
AF = mybir.ActivationFunctionType
ALU = mybir.AluOpType
AX = mybir.AxisListType
BF = ml_dtypes.bfloat16

# packed bf16 params column layout
PC_QT = 0  # cols 0:128    qT chunks (64 cols each)
PC_WF = 128  # cols 128:134  [w_q k0,k1 | w_mem k0,k1 | w_p k0,k1]
PC_WC = 134  # cols 134:136  w_c chunks
PC_WIN = 136  # cols 136:138  w_in chunks
PC_B = 138  # col 138       att_b at row 0
PC_MK = 139  # cols 139:203  maskt = -1e30*(1-mask) at row 0
NPB = PC_MK + LQ


def build_nc():
    nc = bacc.Bacc("TRN2", target_bir_lowering=False, debug=False)

    cba_d = nc.dram_tensor("cba", [128, KD, NCA, 128], BF16, kind="ExternalInput")
    cbb_d = nc.dram_tensor("cbb", [128, KD, 128], BF16, kind="ExternalInput")
    parb_d = nc.dram_tensor("parb", [128, NPB], BF16, kind="ExternalInput")
    g_d = nc.dram_tensor("G", [LC, 4], F32, kind="ExternalOutput")

    with tile.TileContext(nc) as tc:
        with (
            tc.tile_pool(name="singles", bufs=1) as singles,
            tc.tile_pool(name="ps_sc", bufs=2, space="PSUM") as ps_sc,
            tc.tile_pool(name="ps_cw", bufs=1, space="PSUM") as ps_cw,
        ):
            # ---- input DMAs ----
            parb = singles.tile([128, NPB], BF16)
            nc.gpsimd.dma_start(out=parb, in_=parb_d[:, :])
            cba = singles.tile([128, KD, NCA, 128], BF16)
            nc.gpsimd.dma_start(
                out=cba.rearrange("p k c i -> p (k c i)"),
                in_=cba_d.rearrange("p k c i -> p (k c i)"),
            )
            cbb = singles.tile([128, KD, 128], BF16)
            nc.sync.dma_start(
                out=cbb.rearrange("p k i -> p (k i)"),
                in_=cbb_d.rearrange("p k i -> p (k i)"),
            )

            # ---- constants (Pool) + act table warm (Act) ----
            ones1 = singles.tile([1, 128], BF16)
            nc.gpsimd.memset(ones1, 1.0)
            rv66 = singles.tile([1, GW], BF16)
            nc.gpsimd.memset(rv66, 0.0)
            warm1 = singles.tile([1, 1], F32)
            nc.gpsimd.memset(warm1, 0.0)
            warmo = singles.tile([1, 1], F32)
            nc.scalar.activation(warmo, warm1, AF.Exp)

            def qt(k):
                return parb[:, PC_QT + LQ * k : PC_QT + LQ * (k + 1)]

            # scalar-ptr operands must be fp32: convert small weights first
            wf = singles.tile([128, 6], F32)
            nc.gpsimd.tensor_copy(wf, parb[:, PC_WF : PC_WF + 6])
            bf1 = singles.tile([1, 1], F32)
            nc.gpsimd.tensor_copy(bf1, parb[0:1, PC_B : PC_B + 1])

            # ---- rhsA_k [128, 66] = [w_p*qT | w_c | w_in] (Pool) ----
            rhsA = []
            for k in range(KD):
                rhsA_k = singles.tile([128, GW], BF16, tag=f"rhsA{k}", name=f"rhsA{k}")
                nc.gpsimd.tensor_scalar_mul(rhsA_k[:, 0:LQ], qt(k), wf[:, 4 + k : 5 + k])
                nc.gpsimd.tensor_copy(
                    rhsA_k[:, LQ : LQ + 1], parb[:, PC_WC + k : PC_WC + k + 1]
                )
                nc.gpsimd.tensor_copy(
                    rhsA_k[:, LQ + 1 : LQ + 2], parb[:, PC_WIN + k : PC_WIN + k + 1]
                )
                rhsA.append(rhsA_k)

            # ---- sq -> rv (Pool partition_all_reduce; all SBUF) ----
            tq = singles.tile([128, LQ], F32)
            sq_bc = singles.tile([128, LQ], F32)
            nc.gpsimd.tensor_scalar_mul(tq, qt(0), wf[:, 0:1])
            nc.gpsimd.scalar_tensor_tensor(
                tq, qt(1), wf[:, 1:2], tq, op0=ALU.mult, op1=ALU.add
            )
            nc.gpsimd.partition_all_reduce(sq_bc, tq, 128, bass_isa.ReduceOp.add)
            nc.gpsimd.scalar_tensor_tensor(
                rv66[0:1, 0:LQ],
                sq_bc[0:1, :],
                bf1,
                parb[0:1, PC_MK : PC_MK + LQ],
                op0=ALU.add,
                op1=ALU.add,
            )

            # ---- Pool relays chunks 0..2 so PE waits on Pool sems, not on
            # the DMA completion latency ----
            cb2 = singles.tile([128, KD, NCA, 128], BF16)
            for c in range(NCA):
                for k in range(KD):
                    nc.gpsimd.tensor_copy(cb2[:, k, c, :], cba[:, k, c, :])

            # ---- q1 (needed later, by the U numerator) ----
            tq2 = singles.tile([128, LQ], F32)
            q1bc = singles.tile([128, LQ], BF16)
            nc.gpsimd.tensor_scalar_mul(tq2, qt(0), wf[:, 2:3])
            nc.gpsimd.scalar_tensor_tensor(
                tq2, qt(1), wf[:, 3:4], tq2, op0=ALU.mult, op1=ALU.add
            )
            nc.gpsimd.partition_all_reduce(q1bc, tq2, 128, bass_isa.ReduceOp.add)

            # ---- scores: two half PSUM tiles [128, 2*66] ----
            Sh = []
            for h in range(2):
                S = ps_sc.tile([128, 2 * GW], F32, tag="score", name=f"S{h}")
                Sh.append(S.rearrange("p (c w) -> p c w", c=2))

            def clhs(c, k):
                if c < NCA:
                    return cb2[:, k, c, :]
                return cbb[:, k, :]

            def kmm(c, k):
                h, cc = divmod(c, 2)
                nc.tensor.matmul(
                    Sh[h][:, cc, :], clhs(c, k), rhsA[k], start=(k == 0), stop=False
                )

            def bmm(c):
                h, cc = divmod(c, 2)
                nc.tensor.matmul(Sh[h][:, cc, :], ones1, rv66, start=False, stop=True)

            for c in range(NCHUNK):
                kmm(c, 0)
                kmm(c, 1)
                bmm(c)

            # c.w_c / c.w_in in their own PSUM tile (8 tiny N=2 matmuls) so
            # the exps are the S tiles' only readers (PSUM readers serialize)
            cwps = ps_cw.tile([128, NCHUNK, 2], F32, tag="cwps")
            for c in range(NCHUNK):
                for k in range(KD):
                    nc.tensor.matmul(
                        cwps[:, c, :],
                        clhs(c, k),
                        rhsA[k][:, LQ : LQ + 2],
                        start=(k == 0),
                        stop=(k == KD - 1),
                    )

            # ---- softmax_j pipeline over halves ----
            E = singles.tile([128, NCHUNK, LQ], BF16)
            den4 = singles.tile([128, NCHUNK], F32)
            M4 = singles.tile([128, NCHUNK], BF16)
            num4 = singles.tile([128, NCHUNK], F32)
            prods = singles.tile([128, LQ], BF16)
            dscr = singles.tile([128, 2, 32], F32)
            mscr = singles.tile([128, 2, 32], BF16)

            def pool_tree(dst2, src3, scr, op):
                # src3: [128, 2, 64] -> dst2: [128, 2] via contiguous halving
                nc.gpsimd.tensor_tensor(scr, src3[:, :, 0:32], src3[:, :, 32:64], op)
                w = 16
                while w >= 1:
                    a = scr[:, :, 0:w]
                    if w == 1:
                        a = dst2.rearrange("p (c o) -> p c o", o=1)
                    nc.gpsimd.tensor_tensor(a, scr[:, :, 0:w], scr[:, :, w : 2 * w], op)
                    w //= 2

            for h in range(2):
                nc.scalar.activation(
                    E[:, 2 * h : 2 * h + 2, :], Sh[h][:, :, 0:LQ], AF.Exp
                )
            for h in range(2):
                Eh = E[:, 2 * h : 2 * h + 2, :]
                pool_tree(den4[:, 2 * h : 2 * h + 2], Eh, dscr, ALU.add)
                pool_tree(M4[:, 2 * h : 2 * h + 2], Eh, mscr, ALU.max)
            for c in range(NCHUNK):
                nc.vector.tensor_tensor_reduce(
                    out=prods,
                    in0=E[:, c, :],
                    in1=q1bc,
                    scale=1.0,
                    scalar=0.0,
                    op0=ALU.mult,
                    op1=ALU.add,
                    accum_out=num4[:, c : c + 1],
                )

            # cw extraction (DVE; Pool must not touch PSUM per BIR rules, and
            # a Pool copy here would get scheduled ahead of the pallreduces)
            cw = singles.tile([128, NCHUNK, 2], F32)
            nc.vector.tensor_copy(cw, cwps)
            G = singles.tile([128, NCHUNK, 4], F32)

            def gcol(j):
                return G[:, :, j : j + 1].rearrange("p c o -> p (c o)")

            cwc4 = cw[:, :, 0:1].rearrange("p c o -> p (c o)")
            nc.vector.tensor_copy(gcol(0), cw[:, :, 1:2].rearrange("p c o -> p (c o)"))

            # ---- u softmax over i, on Pool after one small Act exp ----
            ecwc4 = singles.tile([128, NCHUNK], F32)
            nc.scalar.activation(ecwc4, cwc4, AF.Exp)
            em4 = singles.tile([128, NCHUNK], F32)
            emc4 = singles.tile([128, NCHUNK], F32)
            em2 = singles.tile([128, 2], F32)
            em2a = singles.tile([128, 2], F32)
            t2 = singles.tile([128, 2], F32)
            nc.gpsimd.tensor_tensor(em4, M4, ecwc4, ALU.mult)
            nc.gpsimd.tensor_tensor(emc4, em4, gcol(0), ALU.mult)
            nc.gpsimd.tensor_tensor(t2, em4[:, 0:2], em4[:, 2:4], ALU.add)
            nc.gpsimd.tensor_tensor(em2[:, 0:1], t2[:, 0:1], t2[:, 1:2], ALU.add)
            nc.gpsimd.tensor_tensor(t2, emc4[:, 0:2], emc4[:, 2:4], ALU.add)
            nc.gpsimd.tensor_tensor(em2[:, 1:2], t2[:, 0:1], t2[:, 1:2], ALU.add)
            nc.gpsimd.partition_all_reduce(em2a, em2, 128, bass_isa.ReduceOp.add)
            h1 = singles.tile([128, 1], F32)
            nc.gpsimd.tensor_tensor(h1, em2a[:, 1:2], em2a[:, 0:1], ALU.divide)

            nc.gpsimd.tensor_tensor(gcol(1), num4, den4, ALU.divide)  # U
            nc.gpsimd.tensor_tensor(gcol(2), gcol(0), gcol(1), ALU.mult)
            nc.gpsimd.tensor_scalar_mul(gcol(3), gcol(1), h1)

            nc.sync.dma_start(out=g_d.rearrange("(c p) g -> p c g", p=128), in_=G)

    nc.finalize()
    return nc


_NC = None


def _get_nc():
    global _NC
    if _NC is None:
        _NC = build_nc()
    return _NC


def pack_params(att_w, att_b, w_in, w_mem, mask_b, question_b):
    par = np.zeros((128, NPB), np.float32)
    qt = question_b.T.reshape(KD, 128, LQ)
    for k in range(KD):
        par[:, PC_QT + LQ * k : PC_QT + LQ * (k + 1)] = qt[k]
        par[:, PC_WF + k] = att_w[256 + 128 * k : 256 + 128 * (k + 1)]  # w_q
        par[:, PC_WF + 2 + k] = w_mem[128 * k : 128 * (k + 1)]
        par[:, PC_WF + 4 + k] = att_w[512 + 128 * k : 512 + 128 * (k + 1)]  # w_p
        par[:, PC_WC + k] = att_w[128 * k : 128 * (k + 1)]
        par[:, PC_WIN + k] = w_in[128 * k : 128 * (k + 1)]
    par[0, PC_B] = att_b[0]
    par[0, PC_MK : PC_MK + LQ] = -NEG_BIG * (1.0 - mask_b.astype(np.float32))
    return par.astype(BF)


def make_in_maps(context, question, mask, att_w, att_b, w_in, w_mem):
    context = np.asarray(context, np.float32)
    question = np.asarray(question, np.float32)
    mask = np.asarray(mask, np.int32)
    att_w = np.asarray(att_w, np.float32)
    att_b = np.asarray(att_b, np.float32)
    w_in = np.asarray(w_in, np.float32)
    w_mem = np.asarray(w_mem, np.float32)
    maps = []
    for b in range(B):
        # cfull[p, k, c, i'] = context[b, 128*c + i', 128*k + p]
        cfull = context[b].T.reshape(KD, 128, NCHUNK, 128)  # [k, p, c, i']
        cfull = np.moveaxis(cfull, 1, 0)  # [p, k, c, i']
        maps.append(
            {
                "cba": np.ascontiguousarray(cfull[:, :, 0:NCA, :]).astype(BF),
                "cbb": np.ascontiguousarray(cfull[:, :, NCA, :]).astype(BF),
                "parb": pack_params(att_w, att_b, w_in, w_mem, mask[b], question[b]),
            }
        )
    return maps


def kernel(context, question, mask, att_w, att_b, w_in, w_mem):
    nc = _get_nc()
    in_maps = make_in_maps(context, question, mask, att_w, att_b, w_in, w_mem)
    res = run_bass_kernel_spmd(nc, in_maps, core_ids=list(range(B)))
    return np.stack([res.results[c]["G"] for c in range(B)], axis=0)
